# revision 1
# baseline (speedup 1.0000x reference)
"""Trainium2 Bass kernel for the guided-diffusion AttentionBlock (fp8 DR).

Shapes (hardcoded, from the problem spec):
  x: (8, 512, 32, 32) fp32, GroupNorm(32), 8 heads (head dim 64), qkv 1x1
  conv (1536x512), proj 1x1 conv (512x512), residual add.

Sharding: pure data-parallel - one batch item per NeuronCore (8 cores).
Weights are replicated; no collectives.

Design (per core, C=512 channels, L=1024 positions), ~167us vs the 257us
fp16 baseline.  The chip duty-throttles the PE to k=4/8 (~1.2GHz) for
most of the dense phase, so streamed PE cycles are the scarce resource:
  - x ships as fp16, weights as fp8e4 (e4m3) x32-scaled on the host so
    their N(0, 0.02) distribution sits in e4m3's normal range.  qkv/av/
    proj matmuls use the PE's fp8 DoubleRow mode: contraction chunk-pairs
    packed as [128, 2, f] tiles, two stationary planes per PE cell, K=256
    per pass at 1 col/cycle (2x fp16).  Scores stay fp16 (K=64 gains
    nothing from fp8: 1 col/cycle either way, and fp8 ldweights is
    slower); the two heads of a pair row-pack into disjoint PE quadrants
    whose streams overlap.
  - GroupNorm: per-half-tile sum (DVE reduce) + sum-sq (ACT Square with
    accumulate) start under the input DMA wall; a one-hot selector matmul
    contracts them to group stats; xn = x*A + B is written straight into
    the fp8 chunk-pair layout, split across ACT and DVE.
  - Softmax: exp runs with a 2^-10 descale folded in.  The elementwise
    load splits across two engines: ACT's Exp table (10 tiles per head-
    pair) and a DVE "fast exp" (6 tiles) that builds the fp8e4m3 BIT
    PATTERN of e^z with one fused multiply-add into an int8 view
    (Mitchell log2 approx, +-3% ripple; softmax normalization cancels
    the common-mode bias and the attention-weighted average dilutes the
    ripple far below the 2e-2 harness tolerance).  vhat carries an
    all-ones column so the av matmul emits the denominator row for free.
  - The denominator row lives on one partition and a single-lane DVE
    reciprocal is ~8us, so it is packed across 128 lanes by an
    SBUF->SBUF DMA, reciprocal'd there (~0.2us), scattered to DRAM and
    broadcast-read back (partition-step-0 reads are DRAM-only), all on
    the idle sync queue, overlapped a full t-half ahead.  The last
    head-pair's final half instead uses a no-DMA path: a K=1 ones-column
    PE matmul broadcasts the raw denom row and DVE builds 1/x with the
    fp32 magic-subtract bit trick.
  - Biases: q/k biases ride the PSUM evacuations (ACT and DVE take one
    t-half each); the v bias folds into the proj bias on the host
    (softmax weights sum to 1 => proj(a + bv) = proj(a) + W_p bv); the
    proj bias is pre-added into the residual x tiles.  The final
    residual is one DVE scalar_tensor_tensor per block:
    (psum * 2^-10) + (x + bp').

Environment notes: (1) the TileContext epilogue's
EVENT_SEMAPHORE_RANGE_CLEAR / ranged-drain crashes this runtime's exec
unit, so clear_and_free_semaphores is replaced with per-semaphore
sem-wr-imm writes carried on gpsimd NOPs; (2) DVE tensor_tensor_reduce
hangs the exec unit (hence ACT Square for the GN sum-sq); (3) gpsimd
tensor ops run ~15x slower than the big engines and cannot touch PSUM -
only one off-critical-path softmax multiply per head-pair lives there.
"""

import math
import sys

if "/opt/trn_rl_repo" not in sys.path:
    sys.path.insert(0, "/opt/trn_rl_repo")

import numpy as np

import concourse.bass as bass
import concourse.bacc as bacc
import concourse.mybir as mybir
import concourse.tile as tile
from concourse.bass_utils import run_bass_kernel_spmd

B, C, H, W = 8, 512, 32, 32
L = H * W               # 1024
N_HEADS = 8
CH = C // N_HEADS       # 64
N_GROUPS = 32
GSIZE = C // N_GROUPS   # 16
CB = C // 128           # 4 channel blocks
NG_BLK = 128 // GSIZE   # 8 groups per channel block
LT = L // 128           # 8 l-tiles
EPS = 1e-5

F32 = mybir.dt.float32
F16 = mybir.dt.float16
F8 = mybir.dt.float8e4
I8 = mybir.dt.int8
AX = mybir.AxisListType
AF = mybir.ActivationFunctionType
ALU = mybir.AluOpType
DR = mybir.MatmulPerfMode.DoubleRow

import os
USE_FEXP = os.environ.get("ANT_NO_FEXP", "") == ""
USE_DR = os.environ.get("ANT_NO_DR", "") == ""
USE_ACTDMA = os.environ.get("ANT_NO_ACTDMA", "") == ""
PHASE = int(os.environ.get("ANT_PHASE", "4"))  # 1 gn, 2 +qkv, 3 +att, 4 full

S = 32.0                 # fp8 weight scale (keeps w ~ N(0,0.02) in e4m3 range)
DESC = 1.0 / (S * S)     # descale for q.k products and proj outputs
FEXP_A = (8.0 / math.log(2.0)) * DESC   # fast-exp slope on raw (x1024) scores
FEXP_B = 56.05                          # fast-exp offset (e4m3 bias, calibrated)


def _patch_sem_clear():
    """Replace the RANGE_CLEAR epilogue with per-sem sem-wr-imm NOPs."""
    if getattr(bass.Bass, "_ant_semclear_patched", False):
        return

    def clear_and_free_semaphores(self, sems):
        if not sems:
            return
        sem_nums = [
            s.num if isinstance(s, bass.SemaphoreHandle) else s for s in sems
        ]
        for num in sem_nums:
            inst = self.gpsimd.nop(nofuse=True)
            si = inst.ins.sync_info
            if si is None:
                si = mybir.SyncInfo(on_wait=[], on_update=[])
                inst.ins.sync_info = si
            si.on_update.append(
                mybir.SyncUpdate(
                    sync_type="semaphore",
                    id=num,
                    update_mode="sem-wr-imm",
                    update_value=0,
                )
            )
        self._state.prepend_free_semaphores(sem_nums)
        for poison_set in self._tile_sem_poison_stack:
            poison_set.update(sem_nums)

    bass.Bass.clear_and_free_semaphores = clear_and_free_semaphores
    bass.Bass._ant_semclear_patched = True


def build_program():
    _patch_sem_clear()
    nc = bacc.Bacc("TRN2", target_bir_lowering=False, debug=False)

    x_d = nc.declare_dram_parameter("x", [C, L], F16, isOutput=False)
    w_d = {}
    for nm in ("wq", "wk", "wv", "wp"):
        # chunk-pair interleaved: row cp*128+p, col i*512+o holds
        # W_T[(2cp+i)*128+p, o]
        w_d[nm] = nc.declare_dram_parameter(nm, [2 * 128, 2 * C], F8, isOutput=False)
    bq_d = nc.declare_dram_parameter("bq", [1, C], F32, isOutput=False)
    bk_d = nc.declare_dram_parameter("bk", [1, C], F32, isOutput=False)
    bp_d = nc.declare_dram_parameter("bp", [1, C], F32, isOutput=False)
    gam_d = nc.declare_dram_parameter("gamma", [CB, 128], F32, isOutput=False)
    bet_d = nc.declare_dram_parameter("beta", [CB, 128], F32, isOutput=False)
    out_d = nc.declare_dram_parameter("out", [C, L], F32, isOutput=True)
    # DRAM bounce for the softmax reciprocal rows: a partition-step-0
    # broadcast read is only legal from DRAM
    recip_d = nc.dram_tensor("recip_scratch", [N_HEADS, L], F16)

    def mm_dr(out, lhsT, rhs, start, stop):
        """DoubleRow fp8 matmul over a [p, 2, f] chunk pair; falls back to
        two plain K=128 matmuls when ANT_NO_DR is set."""
        if USE_DR:
            nc.tensor.matmul(out, lhsT, rhs, start=start, stop=stop,
                             perf_mode=DR)
        else:
            nc.tensor.matmul(out, lhsT[:, 0, :], rhs[:, 0, :],
                             start=start, stop=False)
            nc.tensor.matmul(out, lhsT[:, 1, :], rhs[:, 1, :],
                             start=False, stop=stop)

    # one-hot group selector (channel-in-block -> group-in-block) and its T
    g_np = np.zeros((128, NG_BLK), dtype=np.float32)
    for c in range(128):
        g_np[c, c // GSIZE] = 1.0
    g_d = nc.inline_tensor(g_np, name="gsel")
    gt_d = nc.inline_tensor(np.ascontiguousarray(g_np.T), name="gselT")

    with tile.TileContext(nc) as tc:
        with (
            tc.tile_pool(name="per", bufs=1) as per,      # persistent sbuf
            tc.tile_pool(name="tmp", bufs=2) as tmp,      # transient sbuf
        ):
            # DMA issue engines (one hardware DGE queue each)
            queues = [nc.sync, nc.scalar] if USE_ACTDMA else [nc.sync, nc.sync]

            # ---------- loads ----------
            # x arrives in half-tiles so the GN stat ops can start earlier
            # under the DMA wall
            x_sb = [per.tile([128, L], F16, name=f"x{i}") for i in range(CB)]
            for cb in range(CB):
                for hv in range(2):
                    queues[(2 * cb + hv) % 2].dma_start(
                        out=x_sb[cb][:, hv * 512:(hv + 1) * 512],
                        in_=x_d.ap()[cb * 128:(cb + 1) * 128,
                                     hv * 512:(hv + 1) * 512])

            w_sb = {}
            qi = 0
            for nm in ("wv", "wq", "wk", "wp"):
                w_sb[nm] = [per.tile([128, 2, C], F8, name=f"{nm}{i}")
                            for i in range(2)]
                for cp in range(2):
                    queues[qi % 2].dma_start(
                        out=w_sb[nm][cp].rearrange("p a b -> p (a b)"),
                        in_=w_d[nm].ap()[cp * 128:(cp + 1) * 128, :])
                    qi += 1

            bq_col = per.tile([128, CB], F32, name="bq_col")
            bk_col = per.tile([128, CB], F32, name="bk_col")
            bp_col = per.tile([128, CB], F32, name="bp_col")
            for ob in range(CB):
                nc.sync.dma_start(out=bq_col[:, ob:ob + 1],
                                  in_=bq_d.ap()[0, ob * 128:(ob + 1) * 128])
                nc.sync.dma_start(out=bk_col[:, ob:ob + 1],
                                  in_=bk_d.ap()[0, ob * 128:(ob + 1) * 128])
                nc.sync.dma_start(out=bp_col[:, ob:ob + 1],
                                  in_=bp_d.ap()[0, ob * 128:(ob + 1) * 128])
            gam_sb = per.tile([128, CB], F32, name="gam")
            bet_sb = per.tile([128, CB], F32, name="bet")
            for cb in range(CB):
                nc.sync.dma_start(out=gam_sb[:, cb:cb + 1], in_=gam_d.ap()[cb])
                nc.sync.dma_start(out=bet_sb[:, cb:cb + 1], in_=bet_d.ap()[cb])

            g_sb = per.tile([128, NG_BLK], F32, name="gsel")
            nc.sync.dma_start(out=g_sb, in_=g_d.ap())
            gt_sb = per.tile([NG_BLK, 128], F32, name="gselT")
            nc.sync.dma_start(out=gt_sb, in_=gt_d.ap())

            eps_sb = per.tile([NG_BLK, 1], F32, name="eps")
            nc.vector.memset(eps_sb, EPS)
            ones_col = per.tile([128, CH], F16, name="ones_col")
            nc.vector.memset(ones_col, 1.0)
            # prime the ACT function table while the DMA wall drains
            prime_scr = per.tile([NG_BLK, 1], F32, name="prime_scr")
            nc.scalar.activation(out=prime_scr, in_=eps_sb, func=AF.Square)

            # ---------- GroupNorm ----------
            stats = per.tile([128, 2 * CB], F32, name="stats")
            # xn chunk-pair layout: xn_pair[cp][:, i, :] = xn chunk 2cp+i
            xn_pair = [per.tile([128, 2, L], F8, name=f"xnp{i}") for i in range(2)]
            if PHASE >= 1:
                with tc.tile_pool(name="ps_gn", bufs=1, space="PSUM") as ps_gn:
                    # per-half partial stats (start as each half-tile lands),
                    # then one tiny add folds the halves
                    statp = per.tile([128, 2, 2 * CB], F32, name="statp")
                    for cb in range(CB):
                        for hv in range(2):
                            xh = x_sb[cb][:, hv * 512:(hv + 1) * 512]
                            nc.vector.tensor_reduce(
                                out=statp[:, hv, 2 * cb:2 * cb + 1], in_=xh,
                                axis=AX.X, op=ALU.add,
                            )
                            sq_scr = tmp.tile([128, 512], F16, name="sq_scr",
                                              tag="sq_scr")
                            # NOTE: DVE tensor_tensor_reduce hangs this
                            # runtime's exec unit; ACT Square instead.
                            nc.scalar.activation(
                                out=sq_scr, in_=xh, func=AF.Square,
                                accum_out=statp[:, hv, 2 * cb + 1:2 * cb + 2],
                            )
                    nc.vector.tensor_add(
                        out=stats, in0=statp[:, 0, :], in1=statp[:, 1, :])
                    gstat_ps = ps_gn.tile([NG_BLK, 2 * CB], F32, name="gstat")
                    nc.tensor.matmul(gstat_ps, g_sb, stats, start=True, stop=True)

                    inv_n = 1.0 / (GSIZE * L)
                    mu = tmp.tile([NG_BLK, CB], F32, name="mu", bufs=1)
                    ex2 = tmp.tile([NG_BLK, CB], F32, name="ex2", bufs=1)
                    nc.scalar.mul(out=mu, in_=gstat_ps[:, 0::2], mul=inv_n)
                    nc.scalar.mul(out=ex2, in_=gstat_ps[:, 1::2], mul=inv_n)
                    var = tmp.tile([NG_BLK, CB], F32, name="var", bufs=1)
                    nc.vector.tensor_mul(out=var, in0=mu, in1=mu)
                    nc.vector.tensor_sub(out=var, in0=ex2, in1=var)
                    nc.scalar.activation(out=var, in_=var, func=AF.Sqrt, bias=eps_sb)
                    rs = tmp.tile([NG_BLK, CB], F32, name="rs", bufs=1)
                    nc.vector.reciprocal(out=rs, in_=var)
                    # rhs for the broadcast matmul: cols 2b = rs, 2b+1 = mu*rs
                    rbc = tmp.tile([NG_BLK, 2 * CB], F32, name="rbc", bufs=1)
                    nc.vector.tensor_copy(rbc[:, 0::2], rs)
                    nc.vector.tensor_mul(out=rbc[:, 1::2], in0=mu, in1=rs)
                    chan_ps = ps_gn.tile([128, 2 * CB], F32, name="chan")
                    nc.tensor.matmul(chan_ps, gt_sb, rbc, start=True, stop=True)

                    # per-channel A = rs*gamma ; B = beta - mu*rs*gamma
                    ab = per.tile([128, 2 * CB], F32, name="ab")
                    nc.vector.tensor_mul(out=ab[:, 0::2], in0=chan_ps[:, 0::2], in1=gam_sb)
                    nc.vector.tensor_mul(out=ab[:, 1::2], in0=chan_ps[:, 1::2], in1=gam_sb)
                    nc.vector.tensor_sub(out=ab[:, 1::2], in0=bet_sb, in1=ab[:, 1::2])
                    # xn split across ACT and DVE so the serial chain to
                    # the first qkv matmuls is ~2 ops deep, not 4
                    for cb in range(CB):
                        if cb % 2 == 0:
                            nc.scalar.activation(
                                out=xn_pair[cb // 2][:, cb % 2, :],
                                in_=x_sb[cb], func=AF.Identity,
                                scale=ab[:, 2 * cb:2 * cb + 1],
                                bias=ab[:, 2 * cb + 1:2 * cb + 2],
                            )
                        else:
                            nc.vector.tensor_scalar(
                                out=xn_pair[cb // 2][:, cb % 2, :],
                                in0=x_sb[cb],
                                scalar1=ab[:, 2 * cb:2 * cb + 1],
                                scalar2=ab[:, 2 * cb + 1:2 * cb + 2],
                                op0=ALU.mult, op1=ALU.add,
                            )
                    # residual prep: x_sb <- x + bp' (in place, after xn read x)
                    for cb in range(CB):
                        nc.vector.tensor_scalar_add(
                            out=x_sb[cb], in0=x_sb[cb],
                            scalar1=bp_col[:, cb:cb + 1],
                        )

            if PHASE >= 2:
                # ---------- qkv ----------
                # q/k stay fp16: fp8 buys nothing for the K=64 score matmuls
                # (1 col/cycle either way) and fp8 ldweights is slower
                q_sb = [per.tile([128, L], F16, name=f"q{i}") for i in range(CB)]
                k_sb = [per.tile([128, L], F16, name=f"k{i}") for i in range(CB)]
                # vhat pair p holds s-chunks 2p,2p+1; head h at cols h*65..h*65+63,
                # col h*65+64 is all-ones (softmax denominator trick)
                vhat_pk = [per.tile([128, 2, N_HEADS * (CH + 2)], F8, name=f"vh{i}")
                           for i in range(LT // 2)]
                for p in range(LT // 2):
                    nc.vector.memset(
                        vhat_pk[p].rearrange("q a (h c) -> q a h c", c=CH + 2)[:, :, :, CH:CH + 2],
                        1.0,
                    )
                with tc.tile_pool(name="ps_v", bufs=1, space="PSUM") as ps_v:
                    # cp-outer: the cp0 sweep starts as soon as xn chunks 0,1
                    # exist (saves the wait for the full xn); all 8 v_ps banks
                    # stay resident until their cp1 stop
                    v_ps = [ps_v.tile([128, C], F32, name=f"v_ps{lt}",
                                      tag="v_ps", bufs=8) for lt in range(LT)]
                    for cp in range(2):
                        for lt in range(LT):
                            # one accumulation group per 2KB psum bank
                            for oh in range(2):
                                mm_dr(
                                    v_ps[lt][:, oh * 256:(oh + 1) * 256],
                                    xn_pair[cp][:, :, lt * 128:(lt + 1) * 128],
                                    w_sb["wv"][cp][:, :, oh * 256:(oh + 1) * 256],
                                    start=(oh == 0 and cp == 0),
                                    stop=(oh == 1 and cp == 1),
                                )
                    for lt in range(LT):
                        # interleaved copy into vhat (8 blocks of 64, stride 66)
                        nc.vector.tensor_copy(
                            vhat_pk[lt // 2].rearrange(
                                "q a (h c) -> q a h c", c=CH + 2
                            )[:, lt % 2, :, 0:CH],
                            v_ps[lt].rearrange("q (h c) -> q h c", c=CH),
                        )
                with tc.tile_pool(name="ps_qkv", bufs=1, space="PSUM") as ps_qkv:
                    for ob in range(CB):
                        qk_ps = {}
                        for nm in ("wq", "wk"):
                            qk_ps[nm] = ps_qkv.tile(
                                [128, L], F32, name=f"{nm}_ps",
                                tag=f"{nm}_ps", bufs=1)
                            for tq in range(4):
                                for cp in range(2):
                                    mm_dr(
                                        qk_ps[nm][:, tq * 256:(tq + 1) * 256],
                                        w_sb[nm][cp][:, :, ob * 128:(ob + 1) * 128],
                                        xn_pair[cp][:, :, tq * 256:(tq + 1) * 256],
                                        start=(cp == 0 and tq % 2 == 0),
                                        stop=(cp == 1 and tq % 2 == 1),
                                    )
                        # evacuate each psum on both ACT and DVE (one t-half
                        # each): halves the evac latency so bufs=1 pipelines
                        # q(ob+1) behind k(ob) without a PE stall
                        for dst, src, col in (
                            (q_sb[ob], qk_ps["wq"], bq_col),
                            (k_sb[ob], qk_ps["wk"], bk_col),
                        ):
                            nc.scalar.activation(
                                out=dst[:, 0:512], in_=src[:, 0:512],
                                func=AF.Identity, bias=col[:, ob:ob + 1],
                            )
                            nc.vector.tensor_scalar_add(
                                out=dst[:, 512:1024], in0=src[:, 512:1024],
                                scalar1=col[:, ob:ob + 1],
                            )

            if PHASE >= 3:
                # ---------- attention ----------
                # a_pk: proj rhs layout, chunk-pair interleaved like xn_pair
                a_pk = [per.tile([128, 2, L], F8, name=f"apk{i}") for i in range(2)]
                with tc.tile_pool(name="ps_att", bufs=1, space="PSUM") as ps_att:
                    for hp in range(N_HEADS // 2):
                        aun_sb = {}
                        for sub in range(2):
                            aun_sb[sub] = tmp.tile([CH + 1, L], F16,
                                                   name=f"aunsb{sub}",
                                                   tag=f"aunsb{sub}", bufs=2)
                        for hf in range(2):
                            aun_ps = {}
                            for sub in range(2):
                                aun_ps[sub] = ps_att.tile(
                                    [CH + 2, 512], F32, name=f"aun{sub}",
                                    tag=f"aun{sub}", bufs=1)
                            for sp in range(4):
                                for sub in range(2):
                                    pl = sub * CH
                                    h = hp * 2 + sub
                                    sct = ps_att.tile([128, 2, 512], F32,
                                                      name="sct", tag="sc", bufs=3)
                                    for i in range(2):
                                        st = 2 * sp + i
                                        nc.tensor.matmul(
                                            sct[:, i, :],
                                            k_sb[hp][pl:pl + CH, st * 128:(st + 1) * 128],
                                            q_sb[hp][pl:pl + CH, hf * 512:(hf + 1) * 512],
                                            start=True, stop=True,
                                            tile_position=(pl, 0),
                                        )
                                    ext = tmp.tile([128, 2, 512], F8, name="ext",
                                                   tag=f"ex{sub}", bufs=3)
                                    if USE_FEXP and sub == 1 and sp >= 1:
                                        # DVE fast-exp: build e4m3 bits of e^z
                                        nc.vector.tensor_scalar(
                                            out=ext.bitcast(I8), in0=sct,
                                            scalar1=FEXP_A, scalar2=FEXP_B,
                                            op0=ALU.mult, op1=ALU.add,
                                        )
                                    else:
                                        nc.scalar.activation(
                                            out=ext, in_=sct, func=AF.Exp,
                                            scale=DESC,
                                        )
                                    for tq in range(2):
                                        mm_dr(
                                            aun_ps[sub][:, tq * 256:(tq + 1) * 256],
                                            vhat_pk[sp][:, :, h * (CH + 2):(h + 1) * (CH + 2)],
                                            ext[:, :, tq * 256:(tq + 1) * 256],
                                            start=(sp == 0 and tq == 0),
                                            stop=(sp == 3 and tq == 1),
                                        )
                            # gpsimd cannot read PSUM; split evacuation ACT/DVE
                            nc.scalar.activation(
                                out=aun_sb[0][:, hf * 512:(hf + 1) * 512],
                                in_=aun_ps[0][0:CH + 1, :], func=AF.Identity,
                            )
                            nc.vector.tensor_copy(
                                out=aun_sb[1][:, hf * 512:(hf + 1) * 512],
                                in_=aun_ps[1][0:CH + 1, :],
                            )
                            # softmax division, per t-half so the hf=0 chain
                            # overlaps the hf=1 compute.  The denom row lives
                            # on one partition; a single-lane DVE reciprocal
                            # is ~8us, so pack it across 128 lanes with an
                            # SBUF->SBUF DMA, reciprocal there (~0.2us),
                            # scatter to DRAM, and broadcast-read back
                            # (partition-step-0 is DRAM-only).  All hops ride
                            # the idle sync DMA queue - no PE/PSUM involved.
                            for sub in range(2):
                                h = hp * 2 + sub
                                hsl = slice(hf * 512, (hf + 1) * 512)
                                dst = a_pk[hp // 2][
                                    (h % 2) * CH:(h % 2) * CH + CH,
                                    (hp % 2), hsl]
                                if hp == 3 and hf == 1:
                                    # tail fast path, no DMA hops: PE
                                    # broadcasts the raw denom row into PSUM,
                                    # DVE builds 1/denom with the fp32
                                    # magic-subtract bit trick (+-5%, only
                                    # ever used for this final half-tile),
                                    # then multiplies.
                                    bc_ps = ps_att.tile(
                                        [128, 512], F32, name="bc_ps",
                                        tag="sc", bufs=3)
                                    nc.tensor.matmul(
                                        bc_ps[0:CH, :],
                                        ones_col[CH:CH + 1, :],
                                        aun_sb[sub][CH:CH + 1, hsl],
                                        start=True, stop=True,
                                        tile_position=(CH, 0),
                                    )
                                    rmag = tmp.tile([CH, 512], F32,
                                                    name="rmag",
                                                    tag=f"rm{sub}", bufs=1)
                                    nc.vector.tensor_scalar(
                                        out=rmag.bitcast(mybir.dt.int32),
                                        in0=bc_ps[0:CH, :].bitcast(
                                            mybir.dt.int32),
                                        scalar1=-1, scalar2=0x7EF31000,
                                        op0=ALU.mult, op1=ALU.add,
                                    )
                                    nc.vector.tensor_mul(
                                        out=dst, in0=aun_sb[sub][0:CH, hsl],
                                        in1=rmag)
                                    continue
                                dpack = tmp.tile([128, LT // 2], F16,
                                                 name="dpack",
                                                 tag=f"dp{sub}", bufs=2)
                                nc.sync.dma_start(
                                    out=dpack, in_=aun_sb[sub][CH:CH + 1, hsl])
                                with nc.allow_low_precision(
                                        reason="1/denom f16: denom<4e3"):
                                    nc.vector.reciprocal(out=dpack, in_=dpack)
                                nc.sync.dma_start(
                                    out=recip_d.ap()[h:h + 1, hsl], in_=dpack)
                                bcast = tmp.tile([CH, 512], F16, name="bcast",
                                                 tag=f"bc{sub}", bufs=2)
                                src = recip_d.ap()[h:h + 1, hsl]
                                src = bass.AP(
                                    tensor=src.tensor, offset=src.offset,
                                    ap=[[0, CH], [1, 512]],
                                )
                                nc.sync.dma_start(out=bcast, in_=src)
                                mul_eng = nc.gpsimd if sub == 0 else nc.vector
                                mul_eng.tensor_mul(
                                    out=dst, in0=aun_sb[sub][0:CH, hsl],
                                    in1=bcast)

            if PHASE >= 4:
                # ---------- proj + residual ----------
                with tc.tile_pool(name="ps_prj", bufs=1, space="PSUM") as ps_prj:
                    o_ps = [ps_prj.tile([128, L], F32, name=f"o{ob}", bufs=1)
                            for ob in range(CB)]
                    for cp in range(2):
                        for ob in range(CB):
                            for tq in range(4):
                                mm_dr(
                                    o_ps[ob][:, tq * 256:(tq + 1) * 256],
                                    w_sb["wp"][cp][:, :, ob * 128:(ob + 1) * 128],
                                    a_pk[cp][:, :, tq * 256:(tq + 1) * 256],
                                    start=(cp == 0 and tq % 2 == 0),
                                    stop=(cp == 1 and tq % 2 == 1),
                                )
                    for ob in range(CB):
                        res = tmp.tile([128, L], F32, name="res",
                                       tag="res", bufs=4)
                        nc.vector.scalar_tensor_tensor(
                            out=res, in0=o_ps[ob],
                            scalar=DESC, op0=ALU.mult,
                            in1=x_sb[ob], op1=ALU.add,
                        )
                        queues[ob % 2].dma_start(
                            out=out_d.ap()[ob * 128:(ob + 1) * 128, :],
                            in_=res,
                        )

    nc.compile()
    return nc


def make_in_maps(x, gn_scale, gn_bias, qkv_w, qkv_b, proj_w, proj_b):
    scale = 1.0 / math.sqrt(math.sqrt(CH))
    f8 = mybir.dt.np(F8)
    xf = np.asarray(x, dtype=np.float32).reshape(B, C, L)
    qkv_w = np.asarray(qkv_w, dtype=np.float32)
    qkv_b = np.asarray(qkv_b, dtype=np.float32)
    proj_w = np.asarray(proj_w, dtype=np.float32)
    proj_b = np.asarray(proj_b, dtype=np.float32)

    def pack_w(wt):
        # W_T [c_in, c_out] -> [2, 128, 2, c_out] (cp, p, i, o) -> [256, 2*c_out]
        a = np.ascontiguousarray(wt.T).reshape(2, 2, 128, C)
        a = a.transpose(0, 2, 1, 3).reshape(2 * 128, 2 * C)
        return np.ascontiguousarray(a.astype(f8))

    common = {
        "wq": pack_w(qkv_w[0:C] * (scale * S)),
        "wk": pack_w(qkv_w[C:2 * C] * (scale * S)),
        "wv": pack_w(qkv_w[2 * C:3 * C] * S),
        "wp": pack_w(proj_w * S),
        "bq": np.ascontiguousarray((qkv_b[0:C] * (scale * S)).reshape(1, C)),
        "bk": np.ascontiguousarray((qkv_b[C:2 * C] * (scale * S)).reshape(1, C)),
        # v bias folded into proj bias: proj(a + bv) = proj(a) + Wp bv
        "bp": np.ascontiguousarray(
            (proj_b + proj_w @ qkv_b[2 * C:3 * C]).reshape(1, C)),
        "gamma": np.ascontiguousarray(
            np.asarray(gn_scale, dtype=np.float32).reshape(CB, 128)),
        "beta": np.ascontiguousarray(
            np.asarray(gn_bias, dtype=np.float32).reshape(CB, 128)),
    }
    return [
        {"x": np.ascontiguousarray(xf[b].astype(np.float16)), **common}
        for b in range(B)
    ]


def run(inputs, trace=False, trace_kwargs=None):
    nc = build_program()
    in_maps = make_in_maps(**inputs)
    res = run_bass_kernel_spmd(
        nc, in_maps, list(range(B)), trace=trace, **(trace_kwargs or {})
    )
    out = np.stack([res.results[b]["out"] for b in range(B)], axis=0)
    return out.reshape(B, C, H, W), res


def kernel(**inputs):
    out, _ = run(inputs)
    return out

